# revision 7
# baseline (speedup 1.0000x reference)
"""Trainium2 Bass kernel for nn_ComplexProjMeasurement.

Math (reference): for batch j (B=128), output dim i (D=512):
  out[j,i] = kr_i^T R_j kr_i + ki_i^T R_j ki_i - ki_i^T I_j kr_i + kr_i^T I_j ki_i
where kr = kernel[:,:,0], ki = kernel[:,:,1] (rows kr_i = kr[i,:]),
R_j = input_real[j], I_j = input_imag[j].

Two-stream symmetrized factorization (2 matmul streams instead of the
3-stream Karatsuba baseline):
  Only sym(R) and antisym(I) contribute:
    out = kr^T Rs kr + ki^T Rs ki + kr^T (I - I^T) ki,  Rs = (R + R^T)/2.
  With p = kr + ki, m = kr - ki, M1 = Rs - (I - I^T), M2 = Rs:
    X = p @ M1_j,  Y = m @ M2_j
    out[j,i] = sum_b X[i,b]*kr[i,b] - sum_b Y[i,b]*ki[i,b]
  (p^T A p = 0 for antisymmetric A kills the unwanted cross terms; the
  symmetric cross terms cancel between the two streams.)

On-device per (j, m-tile): two PSUM banks accumulate X/Y from 4 bf16
matmuls each ([K=128,M=128,N=512], contracting a over 4 k-tiles); the
diagonal contraction is two fused VectorE tensor_tensor_reduce ops
straight from PSUM, chained through the per-partition init scalar
(second op starts from the first op's accumulator, with -ki folded in
on host).

M1/M2 are precomputed on host (symmetrization is O(B*D^2) elementwise)
and shipped as bf16, halving DMA vs fp32 and enabling full-rate PE.

Sharding: data-parallel over batch; each of the 8 cores handles 16 j's.
"""

import contextlib

import numpy as np
import ml_dtypes

import concourse.bass as bass
import concourse.mybir as mybir
import concourse.tile as tile
from concourse import bacc
from concourse.bass_utils import run_bass_kernel_spmd

BF16 = mybir.dt.bfloat16
F32 = mybir.dt.float32

B = 128          # full batch
D = 512          # embed dim
NCORES = 8
JPC = B // NCORES          # j's per core = 16
JG = 2                     # j-group size (PSUM: 2 banks per j -> 4 banks)
NGROUPS = JPC // JG        # 8 groups per core
KT = D // 128              # 4 k-tiles (contraction over a)
MT = D // 128              # 4 m-tiles (output i)

_cached_nc = None


def _build_nc(repeat=1):
    nc = bacc.Bacc(None, target_bir_lowering=False)

    m1 = nc.dram_tensor("m1", [JPC, D, D], BF16, kind="ExternalInput")
    m2 = nc.dram_tensor("m2", [JPC, D, D], BF16, kind="ExternalInput")
    pT = nc.dram_tensor("pT", [D, D], BF16, kind="ExternalInput")
    mT = nc.dram_tensor("mT", [D, D], BF16, kind="ExternalInput")
    krd = nc.dram_tensor("krd", [D, D], F32, kind="ExternalInput")
    kin = nc.dram_tensor("kin", [D, D], F32, kind="ExternalInput")
    out = nc.dram_tensor("out", [JPC, D], F32, kind="ExternalOutput")

    MUL = mybir.AluOpType.mult
    ADD = mybir.AluOpType.add

    with tile.TileContext(nc) as tc:
        with (
            tc.tile_pool(name="singles", bufs=1) as singles,
            tc.tile_pool(name="m1pool", bufs=2) as m1pool,
            tc.tile_pool(name="m2pool", bufs=2) as m2pool,
            tc.tile_pool(name="scr", bufs=2) as scrp,
            tc.tile_pool(name="accp", bufs=2) as accp,
            tc.tile_pool(name="ps", bufs=1, space="PSUM") as psp,
        ):
            # --- one-time loads -------------------------------------------
            # matmul weights, transposed [a, i] layouts tiled as [p, kt, i]
            pT_s = singles.tile([128, KT, D], BF16, tag="pT")
            mT_s = singles.tile([128, KT, D], BF16, tag="mT")
            for t, src in ((pT_s, pT), (mT_s, mT)):
                nc.sync.dma_start(
                    out=t, in_=src.rearrange("(kt p) i -> p kt i", p=128)
                )
            # diag-contraction vectors, natural [i, b] layout as [p, m, b]
            krd_s = singles.tile([128, MT, D], F32, tag="krd")
            kin_s = singles.tile([128, MT, D], F32, tag="kin")
            for t, src in ((krd_s, krd), (kin_s, kin)):
                nc.sync.dma_start(
                    out=t, in_=src.rearrange("(m p) b -> p m b", p=128)
                )

            out_buf = singles.tile([128, JPC, MT], F32, tag="out_buf")

            # --- main loop ------------------------------------------------
            rep_ctx = (tc.For_i(0, repeat, 1,
                                hint_engines=(mybir.EngineType.PE,
                                              mybir.EngineType.DVE,
                                              mybir.EngineType.SP))
                       if repeat > 1 else contextlib.nullcontext())
            with rep_ctx:
                for jg in range(NGROUPS):
                    xt = [[None] * KT for _ in range(JG)]
                    yt = [[None] * KT for _ in range(JG)]
                    for jj in range(JG):
                        j = jg * JG + jj
                        xtile = m1pool.tile([128, KT, D], BF16, tag=f"x{jj}",
                                            name=f"x{jg}_{jj}")
                        ytile = m2pool.tile([128, KT, D], BF16, tag=f"y{jj}",
                                            name=f"y{jg}_{jj}")
                        for kt in range(KT):
                            xt[jj][kt] = xtile[:, kt, :]
                            yt[jj][kt] = ytile[:, kt, :]
                        # spread the two streams across four DMA queues
                        # (by group parity) so no queue carries >4.2 MB
                        q1 = nc.sync if jg % 2 == 0 else nc.gpsimd
                        q2 = nc.scalar if jg % 2 == 0 else nc.vector
                        if jg == 0:
                            # split first group's loads so the first matmuls
                            # start as soon as one k-tile has landed
                            for kt in range(KT):
                                ksl = slice(kt * 128, (kt + 1) * 128)
                                qa = (nc.sync, nc.scalar,
                                      nc.gpsimd, nc.vector)[kt]
                                qb = (nc.scalar, nc.sync,
                                      nc.vector, nc.gpsimd)[kt]
                                qa.dma_start(out=xt[jj][kt],
                                             in_=m1[j, ksl, :])
                                qb.dma_start(out=yt[jj][kt],
                                             in_=m2[j, ksl, :])
                        else:
                            xview = m1[j].rearrange("(kt p) b -> p kt b", p=128)
                            yview = m2[j].rearrange("(kt p) b -> p kt b", p=128)
                            q1.dma_start(out=xtile, in_=xview)
                            q2.dma_start(out=ytile, in_=yview)

                    for m in range(MT):
                        ms = bass.ts(m, 128)
                        psx = [psp.tile([128, D], F32, tag=f"px_{jj}",
                                        name=f"px_{jg}_{m}_{jj}")
                               for jj in range(JG)]
                        psy = [psp.tile([128, D], F32, tag=f"py_{jj}",
                                        name=f"py_{jg}_{m}_{jj}")
                               for jj in range(JG)]
                        for jj in range(JG):
                            for kt in range(KT):
                                first, last = kt == 0, kt == KT - 1
                                nc.tensor.matmul(
                                    psx[jj][:, :], pT_s[:, kt, ms],
                                    xt[jj][kt], start=first, stop=last)
                                nc.tensor.matmul(
                                    psy[jj][:, :], mT_s[:, kt, ms],
                                    yt[jj][kt], start=first, stop=last)

                        # out[j, m-tile] = rowsum(X*kr) - rowsum(Y*ki)
                        # out[j, m-tile] = rowsum(X*kr + Y*(-ki)):
                        # ScalarE drains PSUM to SBUF, GpSimd multiplies
                        # (no PSUM port), DVE does the fused 2D reduce.
                        for jj in range(JG):
                            j = jg * JG + jj
                            scr = scrp.tile([128, 2, D], F32, tag="scr",
                                            name=f"scr_{jg}_{m}_{jj}")
                            sx = scrp.tile([128, D], F32, tag="sx",
                                           name=f"sx_{jg}_{m}_{jj}")
                            sy = scrp.tile([128, D], F32, tag="sy",
                                           name=f"sy_{jg}_{m}_{jj}")
                            nc.scalar.copy(out=sx[:, :], in_=psx[jj][:, :])
                            nc.scalar.copy(out=sy[:, :], in_=psy[jj][:, :])
                            nc.gpsimd.tensor_mul(
                                scr[:, 0, :], sx[:, :], krd_s[:, m, :])
                            nc.gpsimd.tensor_mul(
                                scr[:, 1, :], sy[:, :], kin_s[:, m, :])
                            nc.vector.tensor_reduce(
                                out=out_buf[:, j, m:m + 1], in_=scr[:, :, :],
                                axis=mybir.AxisListType.XY, op=ADD)

            # --- store ----------------------------------------------------
            nc.sync.dma_start(
                out=out.rearrange("j (m p) -> p j m", p=128),
                in_=out_buf[:, :, :],
            )

    nc.finalize()
    return nc


def _get_nc():
    global _cached_nc
    if _cached_nc is None:
        _cached_nc = _build_nc()
    return _cached_nc


def make_in_maps(input_real, input_imag, kernel):
    R = np.asarray(input_real, dtype=np.float32)
    I = np.asarray(input_imag, dtype=np.float32)
    Rt = R.transpose(0, 2, 1)
    It = I.transpose(0, 2, 1)
    M2 = 0.5 * (R + Rt)
    M1 = (M2 - I + It).astype(ml_dtypes.bfloat16)
    M2 = M2.astype(ml_dtypes.bfloat16)
    kernel = np.asarray(kernel, dtype=np.float32)
    kr = np.ascontiguousarray(kernel[:, :, 0])
    ki = np.ascontiguousarray(kernel[:, :, 1])
    pT = np.ascontiguousarray((kr + ki).T).astype(ml_dtypes.bfloat16)
    mT = np.ascontiguousarray((kr - ki).T).astype(ml_dtypes.bfloat16)
    kin = -ki
    in_maps = []
    for c in range(NCORES):
        sl = slice(c * JPC, (c + 1) * JPC)
        in_maps.append({
            "m1": M1[sl],
            "m2": M2[sl],
            "pT": pT,
            "mT": mT,
            "krd": kr,
            "kin": kin,
        })
    return in_maps


def kernel(input_real, input_imag, kernel):
    nc = _get_nc()
    in_maps = make_in_maps(input_real, input_imag, kernel)
    res = run_bass_kernel_spmd(nc, in_maps, core_ids=list(range(NCORES)))
    return np.concatenate(
        [res.results[c]["out"] for c in range(NCORES)], axis=0
    ).astype(np.float32)
